# revision 22
# baseline (speedup 1.0000x reference)
"""AGC-LSTM Trainium2 kernel (8 NeuronCores, SPMD).

Strategy
--------
The model is dominated by the LSTMCell weights: W_hh is (4H, H) = (15024, 3756)
f32 = 226 MB, streamed through 12 sequential timesteps.  Everything else
(attention stages, cheb conv, layer norm, final readout) is < 1% of the FLOPs
and bytes; those run on the host in numpy.

Device kernel (per core k of 8), tensor-parallel over the gate dimension:
  - H is padded 3756 -> 4096;  core k owns H-rows [512k, 512k+512) of each of
    the 4 gates (i, f, g, o) -> 2048 gate rows per core.
  - Weights are pre-permuted on the host into a streaming layout and cast to
    bf16: WS[k] has 35 contraction chunks (3 xi-chunks of 128 + 32 h-chunks of
    128) x 4 column-groups x 512 output columns.  Resident in SBUF (18 MB).
  - Per step: gates.T is computed with the stationary/streaming-swap + 4x
    column-tiling trick: lhsT = u-chunk (K=128 x M=32 batch) loaded into PE
    column-group j, rhs = weight chunk (128 x 512); the four N=512 matmuls run
    concurrently in the four 128x32 PE column tiles -> full PE utilization
    despite batch = 32.  PSUM layout: P[32j+b, g*128+v] = gate g, batch b,
    h-row 512k + 128j + v.
  - Elementwise (sigmoid/tanh on ScalarE, muls on VectorE) -> h2 (bf16), c (f32
    sbuf resident).
  - h2 shards are 32x32-block-transposed on VectorE (the block-position
    shuffle is absorbed into the host-side weight permutation), AllGathered
    (bf16, 32KB/rank) through HBM bounce buffers, and DMA'd back with plain
    per-block gathers that land directly in lhsT layout.
  - HAM-keepalive dummy matmuls (accumulating into a scratch PSUM bank so DCE
    keeps them) fill the PE pipe during the collective wait so the PE clock
    never re-throttles; a priming AllGather at kernel start absorbs the ~40us
    first-collective ncfw staging cost under the weight load.

The xi inputs (attention-modulated x_t) do not depend on h, so they are
precomputed for all 12 steps on the host and folded into the contraction
(3 extra chunks, bias folded in as a constant-1 row of u).
"""

import os
import numpy as np
import ml_dtypes

BF16NP = ml_dtypes.bfloat16

# model dims
B, N, T, F, K = 32, 24, 12, 3, 3
COUT = 12
DIN = N * COUT + N + 1          # 313
H = DIN * T                     # 3756
LN_EPS = 1e-5

# device layout dims
NCORES = 8
HC = 512                        # H rows per core (padded)
HP = NCORES * HC                # 4096
UXI = 384                       # xi part of u (313 + 1 bias + pad), 3 chunks
NXI = UXI // 128                # 3
NHCH = HP // 128                # 32 h chunks
NCH = NXI + NHCH                # 35 contraction chunks
NGRP = 4                        # PE column groups
NFREE = 512                     # streamed output columns per group

DUMMY_ROUNDS = int(os.environ.get("KERNEL_DUMMY_ROUNDS", "44"))
FORCE_DUMMY_ORDER = os.environ.get("KERNEL_FORCE_DUMMY_ORDER", "1") == "1"


# ----------------------------------------------------------------------------
# host math helpers (float64)
# ----------------------------------------------------------------------------

def _softmax(x, axis):
    m = np.max(x, axis=axis, keepdims=True)
    e = np.exp(x - m)
    return e / np.sum(e, axis=axis, keepdims=True)


def _sig(x):
    return 1.0 / (1.0 + np.exp(-x))


def _host_pre(I):
    """Everything before the LSTM.  Returns s_a, x_in (B,DIN,T), xi (T,B,DIN)."""
    X = I["X"].astype(np.float64)
    x4 = np.transpose(X, (0, 1, 3, 2))                # (B,N,F,T)
    x_flow = X[:, 0:1, :, 1]                          # (B,1,T)
    x_rain = X[:, :, :, 0]                            # (B,N,T)
    x_pre = np.concatenate([x_rain, x_flow], axis=1)  # (B,N+1,T)

    # spatial attention
    lhs = np.einsum('bnf,ft->bnt', np.einsum('bnft,t->bnf', x4, I["sa_W1"]), I["sa_W2"])
    rhs = np.einsum('f,bnft->btn', I["sa_W3"], x4)
    S = np.einsum('nm,bmk->bnk', I["sa_Vs"],
                  _sig(np.einsum('bnt,btm->bnm', lhs, rhs) + I["sa_bs"]))
    s_a = _softmax(_sig(_softmax(S, axis=1)), axis=0)         # (B,N,N)

    # temporal attention
    xp = np.transpose(x4, (0, 3, 2, 1))               # (B,T,F,N)
    lhs_t = np.einsum('btf,fn->btn', np.einsum('btfn,n->btf', xp, I["ta_U1"]), I["ta_U2"])
    rhs_t = np.einsum('f,bnft->bnt', I["ta_U3"], x4)
    E = np.einsum('tu,buv->btv', I["ta_Ve"],
                  _sig(np.einsum('btn,bnu->btu', lhs_t, rhs_t) + I["ta_be"]))
    t_a = _softmax(_sig(_softmax(E, axis=1)), axis=0)         # (B,T,T)

    # cheb conv (with the reference's reshape-not-transpose)
    xg = X.reshape(B, N, F, -1)                       # (B,N,F,T)
    Tk_at = I["cheb"][None] * s_a[:, None]            # (B,K,N,N)
    rhs_g = np.einsum('bkmn,bmft->bknft', Tk_at, xg)
    xc = np.einsum('bknft,kfo->bnot', rhs_g, I["theta"]) + I["cheb_bias"][:, None]
    xc = np.maximum(xc, 0.0)                          # (B,N,COUT,T)

    mu = xc.mean(-1, keepdims=True)
    var = xc.var(-1, keepdims=True)
    x_ln = (xc - mu) / np.sqrt(var + LN_EPS) * I["ln_g"] + I["ln_b"]

    x_gc = x_ln.reshape(B, -1, 12)                    # (B, N*COUT, 12)
    x_gc_t = np.einsum('bij,bjk->bik', x_gc, t_a)
    x_res = np.concatenate([x_gc_t, x_pre], axis=1)   # (B, DIN, 12)
    x_in = x_res @ I["layer_in_w"].T + I["layer_in_b"]  # (B, DIN, T)

    # per-step attention on x_t (independent of h)
    xs = np.transpose(x_in, (2, 0, 1))                # (T,B,DIN)
    alpha = _softmax(_sig(xs @ I["sa2_w"].T + I["sa2_b"]), axis=2)
    xi = xs * alpha + xs                              # (T,B,DIN)
    return s_a, x_in, xi


def _host_post(I, hs, x_in):
    """hs: (T,B,H) float; returns final out (B,1)."""
    total_ht = np.transpose(hs, (1, 0, 2)).reshape(B, -1)     # (B, T*H)
    beta = _softmax(np.maximum(total_ht @ I["ta2_w"].astype(np.float64).T
                               + I["ta2_b"], 0.0), axis=1)    # (B,T)
    out = np.einsum('tbh,bt->bh', hs, beta)
    out = out + x_in.reshape(B, -1)
    out = np.maximum(out, 0.0) @ I["out_w"].astype(np.float64).T + I["out_b"]
    return out


# ----------------------------------------------------------------------------
# device input packing
# ----------------------------------------------------------------------------

def _build_device_inputs(I, xi):
    W_ih = I["W_ih"].astype(np.float32)               # (4H, DIN)
    W_hh = I["W_hh"].astype(np.float32)               # (4H, H)
    bias = (I["b_ih"] + I["b_hh"]).astype(np.float32)  # (4H,)
    h0 = I["h0"].astype(np.float32)
    c0 = I["c0"].astype(np.float32)

    # Wall[g, hrow, ucol] : padded gate-row x contraction layout
    Wall = np.zeros((4, HP, UXI + HP), dtype=np.float32)
    Wall[:, :H, 0:DIN] = W_ih.reshape(4, H, DIN)
    Wall[:, :H, DIN] = bias.reshape(4, H)
    Wall[:, :H, UXI:UXI + H] = W_hh.reshape(4, H, H)
    # h-contraction permutation: chunk m=(4k+bb), row kk=(32a+c) reads
    # h-index 512k + 128a + 32bb + c  (matches the DVE 32x32 block-transpose
    # layout of the gathered h -- see _lstm_numpy)
    hperm = np.arange(HP).reshape(NCORES, 4, 4, 32).transpose(0, 2, 1, 3).ravel()
    Wall[:, :, UXI:] = Wall[:, :, UXI:][:, :, hperm]

    # WS[k]: (NCH*NGRP, 128, NFREE) bf16
    WS = np.empty((NCORES, NCH * NGRP, 128, NFREE), dtype=BF16NP)
    for k in range(NCORES):
        blk = Wall[:, k * HC:(k + 1) * HC, :]          # (4, 512, 4480)
        b6 = blk.reshape(4, NGRP, 128, NCH, 128)       # g, j, v, c, kk
        WS[k] = (b6.transpose(3, 1, 4, 0, 2)           # c, j, kk, g, v
                   .reshape(NCH * NGRP, 128, NFREE).astype(BF16NP))

    # XIT: (T, 3, 128, 32) bf16 ; u_xi[t] = [xi_t (313), 1, 0...]
    uxi = np.zeros((T, UXI, B), dtype=np.float32)
    uxi[:, 0:DIN, :] = np.transpose(xi, (0, 2, 1)).astype(np.float32)
    uxi[:, DIN, :] = 1.0
    XIT = uxi.reshape(T, NXI, 128, B).astype(BF16NP)

    # H0T: (128, 1024) bf16 in the block-transposed gather layout:
    # H0T[32a+c, 128k+32bb+d] = h0[d, 512k + 128a + 32bb + c]
    h0p = np.zeros((B, HP), dtype=np.float32)
    h0p[:, :H] = h0
    H0T = (h0p.reshape(B, NCORES, 4, 4, 32)      # d, k, a, bb, c
              .transpose(2, 4, 1, 3, 0)          # a, c, k, bb, d
              .reshape(128, NCORES * 128).astype(BF16NP))

    # C0P[k]: (128, 128) f32 : C0P[k][32j+b, v] = c0[b, 512k+128j+v]
    c0p = np.zeros((B, HP), dtype=np.float32)
    c0p[:, :H] = c0
    c0r = c0p.reshape(B, NCORES, NGRP, 128)
    C0P = np.ascontiguousarray(
        c0r.transpose(1, 2, 0, 3).reshape(NCORES, NGRP * B, 128))
    return WS, XIT, H0T, C0P


def _unpack_hs(HS_cores):
    """HS_cores: list of (T,128,128) bf16 -> hs (T,B,H) float64."""
    hs = np.zeros((T, B, HP), dtype=np.float64)
    for k in range(NCORES):
        r = np.asarray(HS_cores[k]).astype(np.float64).reshape(T, NGRP, B, 128)
        hs[:, :, k * HC:(k + 1) * HC] = r.transpose(0, 2, 1, 3).reshape(T, B, HC)
    return hs[:, :, :H]


# ----------------------------------------------------------------------------
# numpy simulation of the device graph (layout validation)
# ----------------------------------------------------------------------------

def _lstm_numpy(WS, XIT, H0T, C0P, exact=False):
    f32 = np.float32

    def cvt(x):
        return x.astype(f32) if not exact else x.astype(f32)

    HT = H0T.astype(f32)                              # (128, 1024)
    Cs = [C0P[k].astype(f32) for k in range(NCORES)]
    HS = [np.zeros((T, 128, 128), dtype=BF16NP) for _ in range(NCORES)]
    for t in range(T):
        h2bf = []
        for k in range(NCORES):
            P = np.zeros((128, NFREE), dtype=f32)
            for c in range(NCH):
                if c < NXI:
                    lhsT = XIT[t, c].astype(f32)      # (128,32)
                else:
                    lhsT = HT[:, (c - NXI) * 32:(c - NXI) * 32 + 32]
                for j in range(NGRP):
                    rhs = WS[k][c * NGRP + j].astype(f32)
                    P[32 * j:32 * j + 32, :] += lhsT.T @ rhs
            si = _sig(P[:, 0:128])
            sf = _sig(P[:, 128:256])
            tg = np.tanh(P[:, 256:384])
            so = _sig(P[:, 384:512])
            c2 = sf * Cs[k] + si * tg
            Cs[k] = c2
            h2 = (so * np.tanh(c2))
            h2b = h2.astype(BF16NP)
            HS[k][t] = h2b
            # DVE 32x32 block transpose: D[32a+c, 32bb+d] = h2b[32a+d, 32bb+c]
            D = (h2b.reshape(4, 32, 4, 32).transpose(0, 3, 2, 1)
                    .reshape(128, 128))
            h2bf.append(D)
        # AllGather of block-transposed shards; plain concat along free dim
        HT = np.concatenate(h2bf, axis=1).astype(f32)  # (128, 1024)
    return HS


# ----------------------------------------------------------------------------
# bass device kernel
# ----------------------------------------------------------------------------

_CACHE = {}


def _build_bass():
    import concourse.bass as bass
    import concourse.mybir as mybir
    from concourse import bacc, tile

    BF16 = mybir.dt.bfloat16
    F32 = mybir.dt.float32
    AF = mybir.ActivationFunctionType

    nc = bacc.Bacc("TRN2", target_bir_lowering=False, debug=False,
                   num_devices=NCORES)
    dWS = nc.dram_tensor("WS", [NCH * NGRP, 128, NFREE], BF16, kind="ExternalInput")
    dXIT = nc.dram_tensor("XIT", [T, NXI, 128, B], BF16, kind="ExternalInput")
    dH0T = nc.dram_tensor("H0T", [128, NCORES * 128], BF16, kind="ExternalInput")
    dC0P = nc.dram_tensor("C0P", [NGRP * B, 128], F32, kind="ExternalInput")
    dHS = nc.dram_tensor("HS", [T, 128, 128], BF16, kind="ExternalOutput")
    dDBG = nc.dram_tensor("DBG", [128, 1], F32, kind="ExternalOutput")

    rg = [list(range(NCORES))]

    with tile.TileContext(nc) as tc:
        with (
            tc.tile_pool(name="wpool", bufs=1) as wpool,
            tc.tile_pool(name="state", bufs=1) as state,
            tc.tile_pool(name="work", bufs=2) as work,
            tc.tile_pool(name="ps", bufs=2, space="PSUM") as ps,
            tc.tile_pool(name="psd", bufs=1, space="PSUM") as psd,
            tc.tile_pool(name="dram", bufs=2, space="DRAM") as dram,
        ):
            WS_s = wpool.tile([128, NCH * NGRP * NFREE], BF16)
            XIT_s = state.tile([128, T * NXI * B], BF16)
            HT = state.tile([128, NCORES * 128], BF16)
            C = state.tile([128, 128], F32)
            PD = psd.tile([128, NFREE], F32)            # HAM dummy target

            # --- prime the collective path while weights load ---
            # the first two ncfw AllGathers cost ~49us and ~24us (descriptor
            # staging warm-up); issue them on scratch buffers so that cost
            # overlaps the weight DMAs instead of stalling steps 0-1
            for w in range(1):
                pagi = dram.tile([128, 128], BF16, name=f"pagi{w}", tag="pagi")
                pago = dram.tile([NCORES * 128, 128], BF16,
                                 addr_space="Shared", name=f"pago{w}", tag="pago")
                nc.gpsimd.collective_compute(
                    "AllGather", mybir.AluOpType.bypass,
                    replica_groups=rg, ins=[pagi.opt()], outs=[pago.opt()])

            # --- loads ---
            nc.sync.dma_start(
                XIT_s[:].rearrange("k (t c b) -> k t c b", t=T, c=NXI),
                dXIT[:].rearrange("t c k b -> k t c b"))
            nc.sync.dma_start(HT[:], dH0T[:])
            nc.sync.dma_start(C[:], dC0P[:])
            for c in range(NCH):
                nc.sync.dma_start(
                    WS_s[:, c * NGRP * NFREE:(c + 1) * NGRP * NFREE]
                        .rearrange("k (j n) -> k j n", j=NGRP),
                    dWS[c * NGRP:(c + 1) * NGRP].rearrange("j k n -> k j n"))

            def lhsT_of(t, c):
                if c < NXI:
                    o = (t * NXI + c) * B
                    return XIT_s[:, o:o + B]
                o = (c - NXI) * 32
                return HT[:, o:o + 32]

            for t in range(T):
                P = ps.tile([128, NFREE], F32, name=f"P{t}", tag="P")
                h_mms = []
                for c in range(NCH):
                    for j in range(NGRP):
                        mm = nc.tensor.matmul(
                            P[32 * j:32 * j + 32, :],
                            lhsT=lhsT_of(t, c),
                            rhs=WS_s[:, (c * NGRP + j) * NFREE:(c * NGRP + j + 1) * NFREE],
                            start=(c == 0), stop=(c == NCH - 1),
                            tile_position=(0, 32 * j),
                        )
                        if c >= NXI:
                            h_mms.append(mm)

                # --- elementwise ---
                SIF = work.tile([128, 256], F32, name=f"SIF{t}", tag="SIF")
                TG = work.tile([128, 128], F32, name=f"TG{t}", tag="TG")
                SO = work.tile([128, 128], F32, name=f"SO{t}", tag="SO")
                T1 = work.tile([128, 128], F32, name=f"T1_{t}", tag="T1")
                T2 = work.tile([128, 128], F32, name=f"T2_{t}", tag="T2")
                TC = work.tile([128, 128], F32, name=f"TC{t}", tag="TC")
                H2B = work.tile([128, 128], BF16, name=f"H2B{t}", tag="H2B")
                nc.scalar.activation(SIF[:], P[:, 0:256], AF.Sigmoid)
                nc.scalar.activation(TG[:], P[:, 256:384], AF.Tanh)
                nc.scalar.activation(SO[:], P[:, 384:512], AF.Sigmoid)
                nc.vector.tensor_mul(T1[:], SIF[:, 128:256], C[:])
                nc.vector.tensor_mul(T2[:], SIF[:, 0:128], TG[:])
                nc.vector.tensor_add(C[:], T1[:], T2[:])
                nc.scalar.activation(TC[:], C[:], AF.Tanh)
                h2mul = nc.vector.tensor_mul(H2B[:], SO[:], TC[:])

                if t < T - 1:
                    # DVE 32x32 block transpose before the AllGather; the W/H0T
                    # host permutation absorbs the block-position shuffle, so
                    # the gathered buffer is a plain concat in lhsT layout.
                    D = work.tile([128, 128], BF16, name=f"D{t}", tag="D")
                    nc.vector.transpose(D[:], H2B[:])
                    agi = dram.tile([128, 128], BF16, name=f"agi{t}", tag="agi")
                    ago = dram.tile([NCORES * 128, 128], BF16,
                                    addr_space="Shared", name=f"ago{t}", tag="ago")
                    agin_dma = nc.sync.dma_start(agi[:], D[:])
                    nc.gpsimd.collective_compute(
                        "AllGather", mybir.AluOpType.bypass,
                        replica_groups=rg, ins=[agi.opt()], outs=[ago.opt()])
                    hs_dma = nc.sync.dma_start(dHS[t], H2B[:])
                    bass._add_dep_helper(hs_dma.ins, agin_dma.ins,
                                         reason="HS write after AG input")
                    for k in range(NCORES):
                        nc.sync.dma_start(
                            HT[:, 128 * k:128 * (k + 1)],
                            ago[128 * k:128 * (k + 1), :])
                else:
                    nc.sync.dma_start(dHS[t], H2B[:])

                if t < T - 1:
                    # --- HAM keep-alive dummies during the collective wait ---
                    # accumulate into PD (never overwritten) so DCE keeps them
                    last_dummy = None
                    for d in range(DUMMY_ROUNDS):
                        for j in range(NGRP):
                            last_dummy = nc.tensor.matmul(
                                PD[32 * j:32 * j + 32, :],
                                lhsT=H2B[:, 0:32],
                                rhs=WS_s[:, j * NFREE:(j + 1) * NFREE],
                                start=(t == 0 and d == 0),
                                stop=(t == T - 2 and d == DUMMY_ROUNDS - 1),
                                skip_group_check=True,
                                tile_position=(0, 32 * j),
                            )
                    if FORCE_DUMMY_ORDER and last_dummy is not None and t + 1 < T:
                        # pin: next step's first h-matmul issues after the dummies
                        tc._dummy_pin = (last_dummy, t)
                        # defer wiring until next step's mms exist
                        if not hasattr(tc, "_pins"):
                            tc._pins = []
                        tc._pins.append((t, last_dummy))

                # wire previous step's dummy pin to this step's first h-mm
                if FORCE_DUMMY_ORDER and hasattr(tc, "_pins") and h_mms:
                    for (tp, dmy) in list(tc._pins):
                        if tp == t - 1:
                            bass._add_dep_helper(
                                h_mms[0].ins, dmy.ins,
                                reason="h-matmuls after HAM dummies")
                            tc._pins.remove((tp, dmy))

            # consume dummy psum so nothing DCEs it
            dbg = state.tile([128, 1], F32)
            nc.vector.tensor_copy(dbg[:], PD[:, 0:1])
            nc.sync.dma_start(dDBG[:], dbg[:])

    nc.compile()
    return nc


def _ensure_ntff_hook():
    """The agent image's antenv lacks axon_hooks; shim it so trace=True works."""
    import sys
    import types
    try:
        from antenv.axon_hooks import get_axon_ntff_profile_hook  # noqa: F401
        return True
    except ImportError:
        pass
    try:
        import importlib.util
        spec = importlib.util.spec_from_file_location(
            "trn_boot", "/root/.axon_site/trn_agent_boot/trn_boot.py")
        tb = importlib.util.module_from_spec(spec)
        spec.loader.exec_module(tb)
        hook = tb._ntff_profile_via_ctypes("/opt/axon/libaxon_pjrt.so")
        mod = types.ModuleType("antenv.axon_hooks")
        _state = {"hook": hook}
        mod.set_axon_ntff_profile_hook = lambda h: _state.__setitem__("hook", h)
        mod.get_axon_ntff_profile_hook = lambda: _state["hook"]
        import antenv
        antenv.axon_hooks = mod
        sys.modules["antenv.axon_hooks"] = mod
        return hook is not None
    except Exception as e:  # profiling is best-effort
        print(f"ntff hook shim failed: {e}")
        return False


def _run_bass(WS, XIT, H0T, C0P, trace=False, tmpdir=None):
    from concourse import bass_utils
    if trace:
        _ensure_ntff_hook()
    if "nc" not in _CACHE:
        _CACHE["nc"] = _build_bass()
    nc = _CACHE["nc"]
    in_maps = [
        {"WS": WS[k], "XIT": XIT, "H0T": H0T, "C0P": C0P[k]}
        for k in range(NCORES)
    ]
    res = bass_utils.run_bass_kernel_spmd(
        nc, in_maps, core_ids=list(range(NCORES)), trace=trace, tmpdir=tmpdir)
    HS = [res.results[k]["HS"] for k in range(NCORES)]
    return HS, res


# ----------------------------------------------------------------------------
# entry point
# ----------------------------------------------------------------------------

def kernel(**inputs):
    I = {k: np.asarray(v) for k, v in inputs.items()}
    s_a, x_in, xi = _host_pre(I)
    WS, XIT, H0T, C0P = _build_device_inputs(I, xi)

    backend = os.environ.get("KERNEL_BACKEND", "bass")
    if backend == "numpy":
        HS = _lstm_numpy(WS, XIT, H0T, C0P)
    else:
        trace = os.environ.get("KERNEL_TRACE", "0") == "1"
        tmpdir = os.environ.get("KERNEL_TRACE_DIR") or None
        HS, res = _run_bass(WS, XIT, H0T, C0P, trace=trace, tmpdir=tmpdir)
        if trace:
            kernel.last_exec_time_ns = res.exec_time_ns
            kernel.last_results = res

    hs = _unpack_hs(HS)
    out = _host_post(I, hs, x_in)
    return out.astype(np.float32), s_a.astype(np.float32)


# revision 23
# speedup vs baseline: 1.1151x; 1.1151x over previous
"""AGC-LSTM Trainium2 kernel (8 NeuronCores, SPMD).

Strategy
--------
The model is dominated by the LSTMCell weights: W_hh is (4H, H) = (15024, 3756)
f32 = 226 MB, streamed through 12 sequential timesteps.  Everything else
(attention stages, cheb conv, layer norm, final readout) is < 1% of the FLOPs
and bytes; those run on the host in numpy.

Device kernel (per core k of 8), tensor-parallel over the gate dimension:
  - H is padded 3756 -> 4096;  core k owns H-rows [512k, 512k+512) of each of
    the 4 gates (i, f, g, o) -> 2048 gate rows per core.
  - Weights are pre-permuted on the host into a streaming layout and cast to
    bf16: WS[k] has 35 contraction chunks (3 xi-chunks of 128 + 32 h-chunks of
    128) x 4 column-groups x 512 output columns.  Resident in SBUF (18 MB).
  - Per step: gates.T is computed with the stationary/streaming-swap + 4x
    column-tiling trick: lhsT = u-chunk (K=128 x M=32 batch) loaded into PE
    column-group j, rhs = weight chunk (128 x 512); the four N=512 matmuls run
    concurrently in the four 128x32 PE column tiles -> full PE utilization
    despite batch = 32.  PSUM layout: P[32j+b, g*128+v] = gate g, batch b,
    h-row 512k + 128j + v.
  - Elementwise (sigmoid/tanh on ScalarE, muls on VectorE) -> h2 (bf16), c (f32
    sbuf resident).
  - h2 shards are 32x32-block-transposed on VectorE (the block-position
    shuffle is absorbed into the host-side weight permutation), AllGathered
    (bf16, 32KB/rank) through HBM bounce buffers, and DMA'd back with plain
    per-block gathers that land directly in lhsT layout.
  - HAM-keepalive dummy matmuls (accumulating into a scratch PSUM bank so DCE
    keeps them) fill the PE pipe during the collective wait so the PE clock
    never re-throttles; a priming AllGather at kernel start absorbs the ~40us
    first-collective ncfw staging cost under the weight load.

The xi inputs (attention-modulated x_t) do not depend on h, so they are
precomputed for all 12 steps on the host and folded into the contraction
(3 extra chunks, bias folded in as a constant-1 row of u).
"""

import os
import numpy as np
import ml_dtypes

BF16NP = ml_dtypes.bfloat16

# model dims
B, N, T, F, K = 32, 24, 12, 3, 3
COUT = 12
DIN = N * COUT + N + 1          # 313
H = DIN * T                     # 3756
LN_EPS = 1e-5

# device layout dims
NCORES = 8
HC = 512                        # H rows per core (padded)
HP = NCORES * HC                # 4096
UXI = 384                       # xi part of u (313 + 1 bias + pad), 3 chunks
NXI = UXI // 128                # 3
NHCH = HP // 128                # 32 h chunks
NCH = NXI + NHCH                # 35 contraction chunks
NGRP = 4                        # PE column groups
NFREE = 512                     # streamed output columns per group

DUMMY_ROUNDS = int(os.environ.get("KERNEL_DUMMY_ROUNDS", "44"))
FORCE_DUMMY_ORDER = os.environ.get("KERNEL_FORCE_DUMMY_ORDER", "1") == "1"


# ----------------------------------------------------------------------------
# host math helpers (float64)
# ----------------------------------------------------------------------------

def _softmax(x, axis):
    m = np.max(x, axis=axis, keepdims=True)
    e = np.exp(x - m)
    return e / np.sum(e, axis=axis, keepdims=True)


def _sig(x):
    return 1.0 / (1.0 + np.exp(-x))


def _host_pre(I):
    """Everything before the LSTM.  Returns s_a, x_in (B,DIN,T), xi (T,B,DIN)."""
    X = I["X"].astype(np.float64)
    x4 = np.transpose(X, (0, 1, 3, 2))                # (B,N,F,T)
    x_flow = X[:, 0:1, :, 1]                          # (B,1,T)
    x_rain = X[:, :, :, 0]                            # (B,N,T)
    x_pre = np.concatenate([x_rain, x_flow], axis=1)  # (B,N+1,T)

    # spatial attention
    lhs = np.einsum('bnf,ft->bnt', np.einsum('bnft,t->bnf', x4, I["sa_W1"]), I["sa_W2"])
    rhs = np.einsum('f,bnft->btn', I["sa_W3"], x4)
    S = np.einsum('nm,bmk->bnk', I["sa_Vs"],
                  _sig(np.einsum('bnt,btm->bnm', lhs, rhs) + I["sa_bs"]))
    s_a = _softmax(_sig(_softmax(S, axis=1)), axis=0)         # (B,N,N)

    # temporal attention
    xp = np.transpose(x4, (0, 3, 2, 1))               # (B,T,F,N)
    lhs_t = np.einsum('btf,fn->btn', np.einsum('btfn,n->btf', xp, I["ta_U1"]), I["ta_U2"])
    rhs_t = np.einsum('f,bnft->bnt', I["ta_U3"], x4)
    E = np.einsum('tu,buv->btv', I["ta_Ve"],
                  _sig(np.einsum('btn,bnu->btu', lhs_t, rhs_t) + I["ta_be"]))
    t_a = _softmax(_sig(_softmax(E, axis=1)), axis=0)         # (B,T,T)

    # cheb conv (with the reference's reshape-not-transpose)
    xg = X.reshape(B, N, F, -1)                       # (B,N,F,T)
    Tk_at = I["cheb"][None] * s_a[:, None]            # (B,K,N,N)
    rhs_g = np.einsum('bkmn,bmft->bknft', Tk_at, xg)
    xc = np.einsum('bknft,kfo->bnot', rhs_g, I["theta"]) + I["cheb_bias"][:, None]
    xc = np.maximum(xc, 0.0)                          # (B,N,COUT,T)

    mu = xc.mean(-1, keepdims=True)
    var = xc.var(-1, keepdims=True)
    x_ln = (xc - mu) / np.sqrt(var + LN_EPS) * I["ln_g"] + I["ln_b"]

    x_gc = x_ln.reshape(B, -1, 12)                    # (B, N*COUT, 12)
    x_gc_t = np.einsum('bij,bjk->bik', x_gc, t_a)
    x_res = np.concatenate([x_gc_t, x_pre], axis=1)   # (B, DIN, 12)
    x_in = x_res @ I["layer_in_w"].T + I["layer_in_b"]  # (B, DIN, T)

    # per-step attention on x_t (independent of h)
    xs = np.transpose(x_in, (2, 0, 1))                # (T,B,DIN)
    alpha = _softmax(_sig(xs @ I["sa2_w"].T + I["sa2_b"]), axis=2)
    xi = xs * alpha + xs                              # (T,B,DIN)
    return s_a, x_in, xi


def _host_post(I, hs, x_in):
    """hs: (T,B,H) float; returns final out (B,1)."""
    total_ht = np.transpose(hs, (1, 0, 2)).reshape(B, -1)     # (B, T*H)
    beta = _softmax(np.maximum(total_ht @ I["ta2_w"].astype(np.float64).T
                               + I["ta2_b"], 0.0), axis=1)    # (B,T)
    out = np.einsum('tbh,bt->bh', hs, beta)
    out = out + x_in.reshape(B, -1)
    out = np.maximum(out, 0.0) @ I["out_w"].astype(np.float64).T + I["out_b"]
    return out


# ----------------------------------------------------------------------------
# device input packing
# ----------------------------------------------------------------------------

def _build_device_inputs(I, xi):
    W_ih = I["W_ih"].astype(np.float32)               # (4H, DIN)
    W_hh = I["W_hh"].astype(np.float32)               # (4H, H)
    bias = (I["b_ih"] + I["b_hh"]).astype(np.float32)  # (4H,)
    h0 = I["h0"].astype(np.float32)
    c0 = I["c0"].astype(np.float32)

    # Wall[g, hrow, ucol] : padded gate-row x contraction layout
    Wall = np.zeros((4, HP, UXI + HP), dtype=np.float32)
    Wall[:, :H, 0:DIN] = W_ih.reshape(4, H, DIN)
    Wall[:, :H, DIN] = bias.reshape(4, H)
    Wall[:, :H, UXI:UXI + H] = W_hh.reshape(4, H, H)
    # h-contraction permutation: chunk m=(4k+bb), row kk=(32a+c) reads
    # h-index 512k + 128a + 32bb + c  (matches the DVE 32x32 block-transpose
    # layout of the gathered h -- see _lstm_numpy)
    hperm = np.arange(HP).reshape(NCORES, 4, 4, 32).transpose(0, 2, 1, 3).ravel()
    Wall[:, :, UXI:] = Wall[:, :, UXI:][:, :, hperm]

    # WS[k]: (NCH*NGRP, 128, NFREE) bf16
    WS = np.empty((NCORES, NCH * NGRP, 128, NFREE), dtype=BF16NP)
    for k in range(NCORES):
        blk = Wall[:, k * HC:(k + 1) * HC, :]          # (4, 512, 4480)
        b6 = blk.reshape(4, NGRP, 128, NCH, 128)       # g, j, v, c, kk
        WS[k] = (b6.transpose(3, 1, 4, 0, 2)           # c, j, kk, g, v
                   .reshape(NCH * NGRP, 128, NFREE).astype(BF16NP))

    # XIT: (T, 3, 128, 32) bf16 ; u_xi[t] = [xi_t (313), 1, 0...]
    uxi = np.zeros((T, UXI, B), dtype=np.float32)
    uxi[:, 0:DIN, :] = np.transpose(xi, (0, 2, 1)).astype(np.float32)
    uxi[:, DIN, :] = 1.0
    XIT = uxi.reshape(T, NXI, 128, B).astype(BF16NP)

    # H0T: (128, 1024) bf16 in the block-transposed gather layout:
    # H0T[32a+c, 128k+32bb+d] = h0[d, 512k + 128a + 32bb + c]
    h0p = np.zeros((B, HP), dtype=np.float32)
    h0p[:, :H] = h0
    H0T = (h0p.reshape(B, NCORES, 4, 4, 32)      # d, k, a, bb, c
              .transpose(2, 4, 1, 3, 0)          # a, c, k, bb, d
              .reshape(128, NCORES * 128).astype(BF16NP))

    # C0P[k]: (128, 128) f32 : C0P[k][32j+b, v] = c0[b, 512k+128j+v]
    c0p = np.zeros((B, HP), dtype=np.float32)
    c0p[:, :H] = c0
    c0r = c0p.reshape(B, NCORES, NGRP, 128)
    C0P = np.ascontiguousarray(
        c0r.transpose(1, 2, 0, 3).reshape(NCORES, NGRP * B, 128))
    return WS, XIT, H0T, C0P


def _unpack_hs(HS_cores):
    """HS_cores: list of (T,128,128) bf16 -> hs (T,B,H) float64."""
    hs = np.zeros((T, B, HP), dtype=np.float64)
    for k in range(NCORES):
        r = np.asarray(HS_cores[k]).astype(np.float64).reshape(T, NGRP, B, 128)
        hs[:, :, k * HC:(k + 1) * HC] = r.transpose(0, 2, 1, 3).reshape(T, B, HC)
    return hs[:, :, :H]


# ----------------------------------------------------------------------------
# numpy simulation of the device graph (layout validation)
# ----------------------------------------------------------------------------

def _lstm_numpy(WS, XIT, H0T, C0P, exact=False):
    f32 = np.float32

    def cvt(x):
        return x.astype(f32) if not exact else x.astype(f32)

    HT = H0T.astype(f32)                              # (128, 1024)
    Cs = [C0P[k].astype(f32) for k in range(NCORES)]
    HS = [np.zeros((T, 128, 128), dtype=BF16NP) for _ in range(NCORES)]
    for t in range(T):
        h2bf = []
        for k in range(NCORES):
            P = np.zeros((128, NFREE), dtype=f32)
            for c in range(NCH):
                if c < NXI:
                    lhsT = XIT[t, c].astype(f32)      # (128,32)
                else:
                    lhsT = HT[:, (c - NXI) * 32:(c - NXI) * 32 + 32]
                for j in range(NGRP):
                    rhs = WS[k][c * NGRP + j].astype(f32)
                    P[32 * j:32 * j + 32, :] += lhsT.T @ rhs
            si = _sig(P[:, 0:128])
            sf = _sig(P[:, 128:256])
            tg = np.tanh(P[:, 256:384])
            so = _sig(P[:, 384:512])
            c2 = sf * Cs[k] + si * tg
            Cs[k] = c2
            h2 = (so * np.tanh(c2))
            h2b = h2.astype(BF16NP)
            HS[k][t] = h2b
            # DVE 32x32 block transpose: D[32a+c, 32bb+d] = h2b[32a+d, 32bb+c]
            D = (h2b.reshape(4, 32, 4, 32).transpose(0, 3, 2, 1)
                    .reshape(128, 128))
            h2bf.append(D)
        # AllGather of block-transposed shards; plain concat along free dim
        HT = np.concatenate(h2bf, axis=1).astype(f32)  # (128, 1024)
    return HS


# ----------------------------------------------------------------------------
# bass device kernel
# ----------------------------------------------------------------------------

_CACHE = {}


def _build_bass():
    import concourse.bass as bass
    import concourse.mybir as mybir
    from concourse import bacc, tile

    BF16 = mybir.dt.bfloat16
    F32 = mybir.dt.float32
    AF = mybir.ActivationFunctionType

    nc = bacc.Bacc("TRN2", target_bir_lowering=False, debug=False,
                   num_devices=NCORES)
    dWS = nc.dram_tensor("WS", [NCH * NGRP, 128, NFREE], BF16, kind="ExternalInput")
    dXIT = nc.dram_tensor("XIT", [T, NXI, 128, B], BF16, kind="ExternalInput")
    dH0T = nc.dram_tensor("H0T", [128, NCORES * 128], BF16, kind="ExternalInput")
    dC0P = nc.dram_tensor("C0P", [NGRP * B, 128], F32, kind="ExternalInput")
    dHS = nc.dram_tensor("HS", [T, 128, 128], BF16, kind="ExternalOutput")
    dDBG = nc.dram_tensor("DBG", [128, 1], F32, kind="ExternalOutput")

    rg = [list(range(NCORES))]

    with tile.TileContext(nc) as tc:
        with (
            tc.tile_pool(name="wpool", bufs=1) as wpool,
            tc.tile_pool(name="state", bufs=1) as state,
            tc.tile_pool(name="work", bufs=2) as work,
            tc.tile_pool(name="ps", bufs=4, space="PSUM") as ps,
            tc.tile_pool(name="psd", bufs=1, space="PSUM") as psd,
            tc.tile_pool(name="dram", bufs=2, space="DRAM") as dram,
        ):
            WS_s = wpool.tile([128, NCH * NGRP * NFREE], BF16)
            XIT_s = state.tile([128, T * NXI * B], BF16)
            HT = state.tile([128, NCORES * 128], BF16)
            C = state.tile([128, 128], F32)
            PD = psd.tile([128, NFREE], F32)            # HAM dummy target

            # --- prime the collective path while weights load ---
            # the first two ncfw AllGathers cost ~49us and ~24us (descriptor
            # staging warm-up); issue them on scratch buffers so that cost
            # overlaps the weight DMAs instead of stalling steps 0-1
            for w in range(1):
                pagi = dram.tile([128, 128], BF16, name=f"pagi{w}", tag="pagi")
                pago = dram.tile([NCORES * 128, 128], BF16,
                                 addr_space="Shared", name=f"pago{w}", tag="pago")
                nc.gpsimd.collective_compute(
                    "AllGather", mybir.AluOpType.bypass,
                    replica_groups=rg, ins=[pagi.opt()], outs=[pago.opt()])

            # --- loads ---
            nc.sync.dma_start(
                XIT_s[:].rearrange("k (t c b) -> k t c b", t=T, c=NXI),
                dXIT[:].rearrange("t c k b -> k t c b"))
            nc.sync.dma_start(HT[:], dH0T[:])
            nc.sync.dma_start(C[:], dC0P[:])
            for c in range(NCH):
                nc.sync.dma_start(
                    WS_s[:, c * NGRP * NFREE:(c + 1) * NGRP * NFREE]
                        .rearrange("k (j n) -> k j n", j=NGRP),
                    dWS[c * NGRP:(c + 1) * NGRP].rearrange("j k n -> k j n"))

            def lhsT_of(t, c):
                if c < NXI:
                    o = (t * NXI + c) * B
                    return XIT_s[:, o:o + B]
                o = (c - NXI) * 32
                return HT[:, o:o + 32]

            for t in range(T):
                P = ps.tile([128, NFREE], F32, name=f"P{t}", tag="P")
                h_mms = []
                for c in range(NCH):
                    for j in range(NGRP):
                        mm = nc.tensor.matmul(
                            P[32 * j:32 * j + 32, :],
                            lhsT=lhsT_of(t, c),
                            rhs=WS_s[:, (c * NGRP + j) * NFREE:(c * NGRP + j + 1) * NFREE],
                            start=(c == 0), stop=(c == NCH - 1),
                            tile_position=(0, 32 * j),
                        )
                        if c >= NXI:
                            h_mms.append(mm)

                # --- elementwise ---
                SIF = work.tile([128, 256], F32, name=f"SIF{t}", tag="SIF")
                TG = work.tile([128, 128], F32, name=f"TG{t}", tag="TG")
                SO = work.tile([128, 128], F32, name=f"SO{t}", tag="SO")
                T1 = work.tile([128, 128], F32, name=f"T1_{t}", tag="T1")
                T2 = work.tile([128, 128], F32, name=f"T2_{t}", tag="T2")
                TC = work.tile([128, 128], F32, name=f"TC{t}", tag="TC")
                H2B = work.tile([128, 128], BF16, name=f"H2B{t}", tag="H2B")
                nc.scalar.activation(SIF[:], P[:, 0:256], AF.Sigmoid)
                nc.scalar.activation(TG[:], P[:, 256:384], AF.Tanh)
                nc.scalar.activation(SO[:], P[:, 384:512], AF.Sigmoid)
                nc.vector.tensor_mul(T1[:], SIF[:, 128:256], C[:])
                nc.vector.tensor_mul(T2[:], SIF[:, 0:128], TG[:])
                nc.vector.tensor_add(C[:], T1[:], T2[:])
                nc.scalar.activation(TC[:], C[:], AF.Tanh)
                h2mul = nc.vector.tensor_mul(H2B[:], SO[:], TC[:])

                if t < T - 1:
                    # DVE 32x32 block transpose before the AllGather; the W/H0T
                    # host permutation absorbs the block-position shuffle, so
                    # the gathered buffer is a plain concat in lhsT layout.
                    D = work.tile([128, 128], BF16, name=f"D{t}", tag="D")
                    nc.vector.transpose(D[:], H2B[:])
                    agi = dram.tile([128, 128], BF16, name=f"agi{t}", tag="agi")
                    ago = dram.tile([NCORES * 128, 128], BF16,
                                    addr_space="Shared", name=f"ago{t}", tag="ago")
                    agin_dma = nc.sync.dma_start(agi[:], D[:])
                    nc.gpsimd.collective_compute(
                        "AllGather", mybir.AluOpType.bypass,
                        replica_groups=rg, ins=[agi.opt()], outs=[ago.opt()])
                    hs_dma = nc.sync.dma_start(dHS[t], H2B[:])
                    bass._add_dep_helper(hs_dma.ins, agin_dma.ins,
                                         reason="HS write after AG input")
                    for k in range(NCORES):
                        nc.sync.dma_start(
                            HT[:, 128 * k:128 * (k + 1)],
                            ago[128 * k:128 * (k + 1), :])
                else:
                    nc.sync.dma_start(dHS[t], H2B[:])

                if t < T - 1:
                    # --- HAM keep-alive dummies during the collective wait ---
                    # accumulate into PD (never overwritten) so DCE keeps them
                    last_dummy = None
                    for d in range(DUMMY_ROUNDS):
                        for j in range(NGRP):
                            last_dummy = nc.tensor.matmul(
                                PD[32 * j:32 * j + 32, :],
                                lhsT=H2B[:, 0:32],
                                rhs=WS_s[:, j * NFREE:(j + 1) * NFREE],
                                start=(t == 0 and d == 0),
                                stop=(t == T - 2 and d == DUMMY_ROUNDS - 1),
                                skip_group_check=True,
                                tile_position=(0, 32 * j),
                            )
                    if FORCE_DUMMY_ORDER and last_dummy is not None and t + 1 < T:
                        # pin: next step's first h-matmul issues after the dummies
                        tc._dummy_pin = (last_dummy, t)
                        # defer wiring until next step's mms exist
                        if not hasattr(tc, "_pins"):
                            tc._pins = []
                        tc._pins.append((t, last_dummy))

                # wire previous step's dummy pin to this step's first h-mm
                if FORCE_DUMMY_ORDER and hasattr(tc, "_pins") and h_mms:
                    for (tp, dmy) in list(tc._pins):
                        if tp == t - 1:
                            bass._add_dep_helper(
                                h_mms[0].ins, dmy.ins,
                                reason="h-matmuls after HAM dummies")
                            tc._pins.remove((tp, dmy))

            # consume dummy psum so nothing DCEs it
            dbg = state.tile([128, 1], F32)
            nc.vector.tensor_copy(dbg[:], PD[:, 0:1])
            nc.sync.dma_start(dDBG[:], dbg[:])

    nc.compile()
    return nc


def _ensure_ntff_hook():
    """The agent image's antenv lacks axon_hooks; shim it so trace=True works."""
    import sys
    import types
    try:
        from antenv.axon_hooks import get_axon_ntff_profile_hook  # noqa: F401
        return True
    except ImportError:
        pass
    try:
        import importlib.util
        spec = importlib.util.spec_from_file_location(
            "trn_boot", "/root/.axon_site/trn_agent_boot/trn_boot.py")
        tb = importlib.util.module_from_spec(spec)
        spec.loader.exec_module(tb)
        hook = tb._ntff_profile_via_ctypes("/opt/axon/libaxon_pjrt.so")
        mod = types.ModuleType("antenv.axon_hooks")
        _state = {"hook": hook}
        mod.set_axon_ntff_profile_hook = lambda h: _state.__setitem__("hook", h)
        mod.get_axon_ntff_profile_hook = lambda: _state["hook"]
        import antenv
        antenv.axon_hooks = mod
        sys.modules["antenv.axon_hooks"] = mod
        return hook is not None
    except Exception as e:  # profiling is best-effort
        print(f"ntff hook shim failed: {e}")
        return False


def _run_bass(WS, XIT, H0T, C0P, trace=False, tmpdir=None):
    from concourse import bass_utils
    if trace:
        _ensure_ntff_hook()
    if "nc" not in _CACHE:
        _CACHE["nc"] = _build_bass()
    nc = _CACHE["nc"]
    in_maps = [
        {"WS": WS[k], "XIT": XIT, "H0T": H0T, "C0P": C0P[k]}
        for k in range(NCORES)
    ]
    res = bass_utils.run_bass_kernel_spmd(
        nc, in_maps, core_ids=list(range(NCORES)), trace=trace, tmpdir=tmpdir)
    HS = [res.results[k]["HS"] for k in range(NCORES)]
    return HS, res


# ----------------------------------------------------------------------------
# entry point
# ----------------------------------------------------------------------------

def kernel(**inputs):
    I = {k: np.asarray(v) for k, v in inputs.items()}
    s_a, x_in, xi = _host_pre(I)
    WS, XIT, H0T, C0P = _build_device_inputs(I, xi)

    backend = os.environ.get("KERNEL_BACKEND", "bass")
    if backend == "numpy":
        HS = _lstm_numpy(WS, XIT, H0T, C0P)
    else:
        trace = os.environ.get("KERNEL_TRACE", "0") == "1"
        tmpdir = os.environ.get("KERNEL_TRACE_DIR") or None
        HS, res = _run_bass(WS, XIT, H0T, C0P, trace=trace, tmpdir=tmpdir)
        if trace:
            kernel.last_exec_time_ns = res.exec_time_ns
            kernel.last_results = res

    hs = _unpack_hs(HS)
    out = _host_post(I, hs, x_in)
    return out.astype(np.float32), s_a.astype(np.float32)
